# revision 17
# baseline (speedup 1.0000x reference)
"""Trainium2 Bass kernel: VALID 2D cross-correlation of [64,1024,1024] fp32
images with a 16x16 kernel -> [64,1009,1009].

Strategy
--------
Data-parallel over batch: 8 images per NeuronCore, 8 cores, no collectives.

Per core, conv is mapped onto the 128x128 TensorE as a banded-Toeplitz
matmul accumulation. For an output row-block of M=113 rows we hold 128
consecutive input rows on SBUF partitions; for each kernel column q
(16 of them) a stationary band matrix W_q[k, m] = kernel[k-m, q]
(nonzero for 0 <= k-m < 16) maps input rows to output rows while the
moving operand is the same x tile column-shifted by q (a free-dim AP
offset, no data movement).  The 16 matmuls accumulate in one PSUM bank:

    out[m, n] = sum_q sum_k W_q[k, m] * x[k, n+q]
              = sum_{p,q} kernel[p, q] * x[m+p, n+q]

Matmuls run in float32r (full-rate fp32 path, ~1.6e-4 max rel err per
128-deep contraction) with fp32 PSUM accumulation.  The Toeplitz
matrices are built on the host in numpy from the 16x16 kernel and
shipped as a tiny [128, 16*113] input.
"""

import numpy as np

B, H, W = 64, 1024, 1024
KH, KW = 16, 16
OH, OW = H - KH + 1, W - KW + 1  # 1009, 1009
NCORES = 8
BPC = B // NCORES  # images per core
MT = 113  # output rows per full row-block (128 - 15)
NBLK = 9  # 8 full blocks + tail block of 105 rows
LAST_M = OH - (NBLK - 1) * MT  # 105
# Column strips: (c0, ncols). 512 + 497 covers OW=1009 exactly.
STRIPS = ((0, 512), (512, OW - 512))

_cache = {}


DTYPE = "fp16"  # matmul operand dtype: "fp16" (fast LDW path) or "f32r"


def _build_nc(reps: int = 1, dtype: str = DTYPE):
    import concourse.mybir as mybir
    import concourse.tile as tile
    from concourse import bacc

    mdt = {"fp16": mybir.dt.float16, "f32r": mybir.dt.float32r}[dtype]
    nc = bacc.Bacc("TRN2", target_bir_lowering=False, debug=False)
    x = nc.dram_tensor("x", [BPC, H, W], mdt, kind="ExternalInput").ap()
    wq = nc.dram_tensor("wq", [128, KW * MT], mdt, kind="ExternalInput").ap()
    y = nc.dram_tensor("y", [BPC, OH, OW], mybir.dt.float32, kind="ExternalOutput").ap()

    with tile.TileContext(nc) as tc:
        with (
            tc.tile_pool(name="w", bufs=1) as wpool,
            tc.tile_pool(name="xp", bufs=8) as xpool,
            tc.tile_pool(name="ps", bufs=7, space="PSUM") as pspool,
            tc.tile_pool(name="op", bufs=4) as opool,
        ):
            w_sb = wpool.tile([128, KW * MT], mdt)
            nc.sync.dma_start(out=w_sb[:, :], in_=wq[:, :])
            for _rep in range(reps):
              for img in range(BPC):
                for b in range(NBLK):
                    r0 = MT * b
                    kk = 128 if b < NBLK - 1 else H - r0  # input rows (120 tail)
                    mm = MT if b < NBLK - 1 else LAST_M  # output rows (105 tail)
                    x_sb = xpool.tile([128, W], mdt)
                    nc.sync.dma_start(out=x_sb[:kk, :], in_=x[img, r0 : r0 + kk, :])
                    # full output row-block staged in SBUF, stored as ONE
                    # contiguous DRAM write (spreads across all 16 SDMA
                    # engines; strided stores pile onto one engine)
                    o_sb = opool.tile([128, OW], mybir.dt.float32)
                    for c0, nn_ in STRIPS:
                        ps = pspool.tile([128, 512], mybir.dt.float32)
                        for q in range(KW):
                            nc.tensor.matmul(
                                ps[:mm, :nn_],
                                lhsT=w_sb[:kk, q * MT : q * MT + mm],
                                rhs=x_sb[:kk, c0 + q : c0 + q + nn_],
                                start=(q == 0),
                                stop=(q == KW - 1),
                            )
                        nc.vector.tensor_copy(
                            o_sb[:mm, c0 : c0 + nn_], ps[:mm, :nn_]
                        )
                    # SWDGE spreads the contiguous store across all 16 SDMA
                    # engines; a single HWDGE store lands on one engine.
                    nc.gpsimd.dma_start(
                        out=y[img, r0 : r0 + mm, :], in_=o_sb[:mm, :]
                    )
    nc.compile()
    return nc


def _toeplitz_weights(k: np.ndarray) -> np.ndarray:
    """wq[kr, q*MT + m] = kernel[kr - m, q] for 0 <= kr - m < KH."""
    wq = np.zeros((128, KW, MT), dtype=np.float32)
    m_idx = np.arange(MT)
    for p in range(KH):
        wq[m_idx + p, :, m_idx] = k[p, :]
    return np.ascontiguousarray(wq.reshape(128, KW * MT))


def _np_dtype():
    return {"fp16": np.float16, "f32r": np.float32}[DTYPE]


def _run(x: np.ndarray, k: np.ndarray, **spmd_kwargs):
    from concourse.bass_utils import run_bass_kernel_spmd

    if "nc" not in _cache:
        _cache["nc"] = _build_nc()
    nc = _cache["nc"]

    ndt = _np_dtype()
    x = np.ascontiguousarray(np.asarray(x, dtype=np.float32).astype(ndt))
    wq_host = _toeplitz_weights(np.asarray(k, dtype=np.float32)).astype(ndt)
    in_maps = [
        {"x": x[i * BPC : (i + 1) * BPC], "wq": wq_host} for i in range(NCORES)
    ]
    return run_bass_kernel_spmd(nc, in_maps, core_ids=list(range(NCORES)), **spmd_kwargs)


def kernel(x: np.ndarray, kernel: np.ndarray) -> np.ndarray:
    try:
        res = _run(x, kernel)
    except Exception:
        # transient NRT/device hiccups are recoverable on retry
        _cache.clear()
        res = _run(x, kernel)
    return np.concatenate([res.results[i]["y"] for i in range(NCORES)], axis=0)


# revision 18
# speedup vs baseline: 1.0003x; 1.0003x over previous
"""Trainium2 Bass kernel: VALID 2D cross-correlation of [64,1024,1024] fp32
images with a 16x16 kernel -> [64,1009,1009].

Strategy
--------
Data-parallel over batch: 8 images per NeuronCore, 8 cores, no collectives.

Per core, conv is mapped onto the 128x128 TensorE as a banded-Toeplitz
matmul accumulation. For an output row-block of M=113 rows we hold 128
consecutive input rows on SBUF partitions; for each kernel column q
(16 of them) a stationary band matrix W_q[k, m] = kernel[k-m, q]
(nonzero for 0 <= k-m < 16) maps input rows to output rows while the
moving operand is the same x tile column-shifted by q (a free-dim AP
offset, no data movement).  The 16 matmuls accumulate in one PSUM bank:

    out[m, n] = sum_q sum_k W_q[k, m] * x[k, n+q]
              = sum_{p,q} kernel[p, q] * x[m+p, n+q]

Matmuls run in fp16 (full-rate PE path, LDWEIGHTS pipelined with
streaming) with fp32 PSUM accumulation; measured ~2.7e-4 scale-relative
max error vs an exact fp64 reference.  The Toeplitz matrices are built
on the host in numpy from the 16x16 kernel and shipped as a tiny
[128, 16*113] input.  Output row-blocks are staged whole in SBUF and
stored as single contiguous DRAM writes via SWDGE (nc.gpsimd), which
spreads each store across all 16 SDMA engines; input loads ride the
HWDGE (nc.sync) queue.  Measured ~527 us per core on TRN2 (8 cores in
parallel over the batch).
"""

import numpy as np

B, H, W = 64, 1024, 1024
KH, KW = 16, 16
OH, OW = H - KH + 1, W - KW + 1  # 1009, 1009
NCORES = 8
BPC = B // NCORES  # images per core
MT = 113  # output rows per full row-block (128 - 15)
NBLK = 9  # 8 full blocks + tail block of 105 rows
LAST_M = OH - (NBLK - 1) * MT  # 105
# Column strips: (c0, ncols). 512 + 497 covers OW=1009 exactly.
STRIPS = ((0, 512), (512, OW - 512))

_cache = {}


DTYPE = "fp16"  # matmul operand dtype: "fp16" (fast LDW path) or "f32r"


def _build_nc(reps: int = 1, dtype: str = DTYPE):
    import concourse.mybir as mybir
    import concourse.tile as tile
    from concourse import bacc

    mdt = {"fp16": mybir.dt.float16, "f32r": mybir.dt.float32r}[dtype]
    nc = bacc.Bacc("TRN2", target_bir_lowering=False, debug=False)
    x = nc.dram_tensor("x", [BPC, H, W], mdt, kind="ExternalInput").ap()
    wq = nc.dram_tensor("wq", [128, KW * MT], mdt, kind="ExternalInput").ap()
    y = nc.dram_tensor("y", [BPC, OH, OW], mybir.dt.float32, kind="ExternalOutput").ap()

    with tile.TileContext(nc) as tc:
        with (
            tc.tile_pool(name="w", bufs=1) as wpool,
            tc.tile_pool(name="xp", bufs=8) as xpool,
            tc.tile_pool(name="ps", bufs=7, space="PSUM") as pspool,
            tc.tile_pool(name="op", bufs=4) as opool,
        ):
            w_sb = wpool.tile([128, KW * MT], mdt)
            nc.sync.dma_start(out=w_sb[:, :], in_=wq[:, :])
            for _rep in range(reps):
              for img in range(BPC):
                for b in range(NBLK):
                    r0 = MT * b
                    kk = 128 if b < NBLK - 1 else H - r0  # input rows (120 tail)
                    mm = MT if b < NBLK - 1 else LAST_M  # output rows (105 tail)
                    x_sb = xpool.tile([128, W], mdt)
                    nc.sync.dma_start(out=x_sb[:kk, :], in_=x[img, r0 : r0 + kk, :])
                    # full output row-block staged in SBUF, stored as ONE
                    # contiguous DRAM write (spreads across all 16 SDMA
                    # engines; strided stores pile onto one engine)
                    o_sb = opool.tile([128, OW], mybir.dt.float32)
                    for c0, nn_ in STRIPS:
                        ps = pspool.tile([128, 512], mybir.dt.float32)
                        for q in range(KW):
                            nc.tensor.matmul(
                                ps[:mm, :nn_],
                                lhsT=w_sb[:kk, q * MT : q * MT + mm],
                                rhs=x_sb[:kk, c0 + q : c0 + q + nn_],
                                start=(q == 0),
                                stop=(q == KW - 1),
                            )
                        nc.vector.tensor_copy(
                            o_sb[:mm, c0 : c0 + nn_], ps[:mm, :nn_]
                        )
                    # SWDGE spreads the contiguous store across all 16 SDMA
                    # engines; a single HWDGE store lands on one engine.
                    nc.gpsimd.dma_start(
                        out=y[img, r0 : r0 + mm, :], in_=o_sb[:mm, :]
                    )
    nc.compile()
    return nc


def _toeplitz_weights(k: np.ndarray) -> np.ndarray:
    """wq[kr, q*MT + m] = kernel[kr - m, q] for 0 <= kr - m < KH."""
    wq = np.zeros((128, KW, MT), dtype=np.float32)
    m_idx = np.arange(MT)
    for p in range(KH):
        wq[m_idx + p, :, m_idx] = k[p, :]
    return np.ascontiguousarray(wq.reshape(128, KW * MT))


def _np_dtype():
    return {"fp16": np.float16, "f32r": np.float32}[DTYPE]


def _run(x: np.ndarray, k: np.ndarray, **spmd_kwargs):
    from concourse.bass_utils import run_bass_kernel_spmd

    if "nc" not in _cache:
        _cache["nc"] = _build_nc()
    nc = _cache["nc"]

    ndt = _np_dtype()
    x = np.ascontiguousarray(np.asarray(x, dtype=np.float32).astype(ndt))
    wq_host = _toeplitz_weights(np.asarray(k, dtype=np.float32)).astype(ndt)
    in_maps = [
        {"x": x[i * BPC : (i + 1) * BPC], "wq": wq_host} for i in range(NCORES)
    ]
    return run_bass_kernel_spmd(nc, in_maps, core_ids=list(range(NCORES)), **spmd_kwargs)


def kernel(x: np.ndarray, kernel: np.ndarray) -> np.ndarray:
    try:
        res = _run(x, kernel)
    except Exception:
        # transient NRT/device hiccups are recoverable on retry
        _cache.clear()
        res = _run(x, kernel)
    return np.concatenate([res.results[i]["y"] for i in range(NCORES)], axis=0)


# revision 19
# speedup vs baseline: 1.0078x; 1.0075x over previous
"""Trainium2 Bass kernel: VALID 2D cross-correlation of [64,1024,1024] fp32
images with a 16x16 kernel -> [64,1009,1009].

Strategy
--------
Data-parallel over batch: 8 images per NeuronCore, 8 cores, no collectives.

Per core, conv is mapped onto the 128x128 TensorE as a banded-Toeplitz
matmul accumulation. For an output row-block of M=113 rows we hold 128
consecutive input rows on SBUF partitions; for each kernel column q
(16 of them) a stationary band matrix W_q[k, m] = kernel[k-m, q]
(nonzero for 0 <= k-m < 16) maps input rows to output rows while the
moving operand is the same x tile column-shifted by q (a free-dim AP
offset, no data movement).  The 16 matmuls accumulate in one PSUM bank:

    out[m, n] = sum_q sum_k W_q[k, m] * x[k, n+q]
              = sum_{p,q} kernel[p, q] * x[m+p, n+q]

Matmuls run in fp16 (full-rate PE path, LDWEIGHTS pipelined with
streaming) with fp32 PSUM accumulation; measured ~2.7e-4 scale-relative
max error vs an exact fp64 reference.  The Toeplitz matrices are built
on the host in numpy from the 16x16 kernel and shipped as a tiny
[128, 16*113] input.  Output row-blocks are staged whole in SBUF and
stored as single contiguous DRAM writes via SWDGE (nc.gpsimd), which
spreads each store across all 16 SDMA engines; input loads ride the
HWDGE (nc.sync) queue.  Measured ~527 us per core on TRN2 (8 cores in
parallel over the batch).
"""

import numpy as np

B, H, W = 64, 1024, 1024
KH, KW = 16, 16
OH, OW = H - KH + 1, W - KW + 1  # 1009, 1009
NCORES = 8
BPC = B // NCORES  # images per core
MT = 113  # output rows per full row-block (128 - 15)
NBLK = 9  # 8 full blocks + tail block of 105 rows
LAST_M = OH - (NBLK - 1) * MT  # 105
# Column strips: (c0, ncols). 512 + 497 covers OW=1009 exactly.
STRIPS = ((0, 512), (512, OW - 512))

_cache = {}


DTYPE = "fp16"  # matmul operand dtype: "fp16" (fast LDW path) or "f32r"


def _build_nc(reps: int = 1, dtype: str = DTYPE):
    import concourse.mybir as mybir
    import concourse.tile as tile
    from concourse import bacc

    mdt = {"fp16": mybir.dt.float16, "f32r": mybir.dt.float32r}[dtype]
    nc = bacc.Bacc("TRN2", target_bir_lowering=False, debug=False)
    x = nc.dram_tensor("x", [BPC, H, W], mdt, kind="ExternalInput").ap()
    wq = nc.dram_tensor("wq", [128, KW * MT], mdt, kind="ExternalInput").ap()
    y = nc.dram_tensor("y", [BPC, OH, OW], mybir.dt.float32, kind="ExternalOutput").ap()

    with tile.TileContext(nc) as tc:
        with (
            tc.tile_pool(name="w", bufs=1) as wpool,
            tc.tile_pool(name="xp", bufs=8) as xpool,
            tc.tile_pool(name="ps", bufs=7, space="PSUM") as pspool,
            tc.tile_pool(name="op", bufs=4) as opool,
        ):
            w_sb = wpool.tile([128, KW * MT], mdt)
            nc.sync.dma_start(out=w_sb[:, :], in_=wq[:, :])
            for _rep in range(reps):
              for img in range(BPC):
                for b in range(NBLK):
                    r0 = MT * b
                    kk = 128 if b < NBLK - 1 else H - r0  # input rows (120 tail)
                    mm = MT if b < NBLK - 1 else LAST_M  # output rows (105 tail)
                    x_sb = xpool.tile([128, W], mdt)
                    if img == 0 and b == 0:
                        # split the very first load by columns so strip-0
                        # matmuls can start before the whole tile lands
                        nc.sync.dma_start(
                            out=x_sb[:kk, :544], in_=x[img, r0 : r0 + kk, :544]
                        )
                        nc.sync.dma_start(
                            out=x_sb[:kk, 544:], in_=x[img, r0 : r0 + kk, 544:]
                        )
                    else:
                        nc.sync.dma_start(
                            out=x_sb[:kk, :], in_=x[img, r0 : r0 + kk, :]
                        )
                    # full output row-block staged in SBUF, stored as ONE
                    # contiguous DRAM write (spreads across all 16 SDMA
                    # engines; strided stores pile onto one engine)
                    o_sb = opool.tile([128, OW], mybir.dt.float32)
                    for c0, nn_ in STRIPS:
                        ps = pspool.tile([128, 512], mybir.dt.float32)
                        for q in range(KW):
                            nc.tensor.matmul(
                                ps[:mm, :nn_],
                                lhsT=w_sb[:kk, q * MT : q * MT + mm],
                                rhs=x_sb[:kk, c0 + q : c0 + q + nn_],
                                start=(q == 0),
                                stop=(q == KW - 1),
                            )
                        nc.vector.tensor_copy(
                            o_sb[:mm, c0 : c0 + nn_], ps[:mm, :nn_]
                        )
                    # SWDGE spreads the contiguous store across all 16 SDMA
                    # engines; a single HWDGE store lands on one engine.
                    nc.gpsimd.dma_start(
                        out=y[img, r0 : r0 + mm, :], in_=o_sb[:mm, :]
                    )
    nc.compile()
    return nc


def _toeplitz_weights(k: np.ndarray) -> np.ndarray:
    """wq[kr, q*MT + m] = kernel[kr - m, q] for 0 <= kr - m < KH."""
    wq = np.zeros((128, KW, MT), dtype=np.float32)
    m_idx = np.arange(MT)
    for p in range(KH):
        wq[m_idx + p, :, m_idx] = k[p, :]
    return np.ascontiguousarray(wq.reshape(128, KW * MT))


def _np_dtype():
    return {"fp16": np.float16, "f32r": np.float32}[DTYPE]


def _run(x: np.ndarray, k: np.ndarray, **spmd_kwargs):
    from concourse.bass_utils import run_bass_kernel_spmd

    if "nc" not in _cache:
        _cache["nc"] = _build_nc()
    nc = _cache["nc"]

    ndt = _np_dtype()
    x = np.ascontiguousarray(np.asarray(x, dtype=np.float32).astype(ndt))
    wq_host = _toeplitz_weights(np.asarray(k, dtype=np.float32)).astype(ndt)
    in_maps = [
        {"x": x[i * BPC : (i + 1) * BPC], "wq": wq_host} for i in range(NCORES)
    ]
    return run_bass_kernel_spmd(nc, in_maps, core_ids=list(range(NCORES)), **spmd_kwargs)


def kernel(x: np.ndarray, kernel: np.ndarray) -> np.ndarray:
    try:
        res = _run(x, kernel)
    except Exception:
        # transient NRT/device hiccups are recoverable on retry
        _cache.clear()
        res = _run(x, kernel)
    return np.concatenate([res.results[i]["y"] for i in range(NCORES)], axis=0)
